# revision 9
# baseline (speedup 1.0000x reference)
"""BilinearPooling Trainium2 kernel.

reference:
    pooled = audio * rowsum(visual)            # [B, D]
    out    = pooled / max(||pooled||_2, eps)   # L2 normalize along D

Since pooled[b,:] = audio[b,:] * s_b with s_b = rowsum(visual[b,:]),
||pooled_b|| = |s_b| * ||audio_b||, so

    out[b,:] = audio[b,:] * s_b / max(|s_b| * ||audio_b||, eps)

Pure data parallel across 8 NeuronCores: batch 8192 -> 1024 rows/core.
Per core: 8 tiles of [128, 2048] f32 (1 MiB per DMA stream), pipelined.
Engine split per tile: rowsum(visual) on DVE, square+accumulate on ACT
(scratch into the dead visual tile), tiny [128,1] scale chain on DVE/ACT,
final scale-multiply on GpSimd (in place in the audio tile). Loads issue
on the SP HWDGE ring, stores on the ACT HWDGE ring so they never queue
behind loads. Memory-bound: 24 MiB/core at ~358 GB/s ≈ 70 us DMA floor.
"""

import numpy as np

import concourse.bass as bass
import concourse.tile as tile
from concourse import mybir
from concourse.bacc import Bacc
from concourse.bass_utils import run_bass_kernel_spmd

B, D = 8192, 2048
N_CORES = 8
ROWS = B // N_CORES          # 1024 rows per core
P = 128                      # SBUF partitions
N_TILES = ROWS // P          # 8
EPS = 1e-12
FP32 = mybir.dt.float32


def build_bass():
    # Bacc (not raw Bass): its finalize() runs the compile pipeline that
    # splits multi-wait instructions into event-semaphore chains — TRN2
    # allows at most one sync wait per instruction, and Tile freely emits
    # more ("Too many sync wait commands" from walrus otherwise).
    nc = Bacc()
    audio = nc.declare_dram_parameter("audio", [ROWS, D], FP32, isOutput=False)
    visual = nc.declare_dram_parameter("visual", [ROWS, D], FP32, isOutput=False)
    out = nc.declare_dram_parameter("out", [ROWS, D], FP32, isOutput=True)

    with tile.TileContext(nc) as tc:
        with (
            tc.tile_pool(name="a_pool", bufs=6) as a_pool,
            tc.tile_pool(name="v_pool", bufs=5) as v_pool,
            tc.tile_pool(name="stats", bufs=8) as stats,
            tc.tile_pool(name="singles", bufs=1) as singles,
        ):
            zero = singles.tile([P, 1], FP32)
            nc.vector.memset(zero, 0.0)

            for t in range(N_TILES):
                r0 = t * P
                a = a_pool.tile([P, D], FP32)
                v = v_pool.tile([P, D], FP32)
                nc.sync.dma_start(out=a, in_=audio[r0 : r0 + P, :])
                nc.sync.dma_start(out=v, in_=visual[r0 : r0 + P, :])

                # s = rowsum(visual)
                s = stats.tile([P, 1], FP32)
                nc.vector.reduce_sum(out=s, in_=v, axis=mybir.AxisListType.X)

                # asq = rowsum(audio^2); full-size squares land in the dead
                # visual tile as scratch.
                asq = stats.tile([P, 1], FP32)
                nc.scalar.activation(
                    out=v,
                    in_=a,
                    func=mybir.ActivationFunctionType.Square,
                    bias=zero,
                    accum_out=asq,
                )

                # sc = s / max(sqrt(s^2 * asq), eps)
                nrm = stats.tile([P, 1], FP32)
                nc.vector.tensor_mul(out=nrm, in0=s, in1=s)
                nc.vector.tensor_mul(out=nrm, in0=nrm, in1=asq)
                nc.scalar.activation(
                    out=nrm,
                    in_=nrm,
                    func=mybir.ActivationFunctionType.Sqrt,
                    bias=zero,
                )
                nc.vector.tensor_scalar_max(out=nrm, in0=nrm, scalar1=EPS)
                nc.vector.reciprocal(out=nrm, in_=nrm)
                sc = stats.tile([P, 1], FP32)
                nc.vector.tensor_mul(out=sc, in0=s, in1=nrm)

                # out = audio * sc, in place. GpSimd is ~14x slower than
                # DVE for full-size f32 tensor_scalar (measured 29 us/op)
                # and its SBUF port traffic stalls DVE — never use it here.
                # Alternate ACT/DVE so consecutive tiles' muls overlap.
                if t % 2 == 0:
                    nc.scalar.mul(out=a, in_=a, mul=sc)
                else:
                    nc.vector.tensor_scalar_mul(out=a, in0=a, scalar1=sc)

                # Store on the ACT HWDGE ring (separate FIFO from loads).
                nc.scalar.dma_start(out=out[r0 : r0 + P, :], in_=a)

    # Runs Bacc.compile() (event-sem wait splitting, reg alloc, ISA
    # codegen) and freezes; run_bass_via_pjrt requires a finalized module.
    nc.finalize()
    return nc


_NC = None


def _get_nc():
    global _NC
    if _NC is None:
        _NC = build_bass()
    return _NC


def kernel(audio: np.ndarray, visual: np.ndarray) -> np.ndarray:
    audio = np.ascontiguousarray(audio, dtype=np.float32)
    visual = np.ascontiguousarray(visual, dtype=np.float32)
    nc = _get_nc()
    in_maps = [
        {
            "audio": audio[i * ROWS : (i + 1) * ROWS],
            "visual": visual[i * ROWS : (i + 1) * ROWS],
        }
        for i in range(N_CORES)
    ]
    res = run_bass_kernel_spmd(nc, in_maps, core_ids=list(range(N_CORES)))
    return np.concatenate([r["out"] for r in res.results], axis=0)
